# revision 43
# baseline (speedup 1.0000x reference)
"""RBF (Gaussian) kernel Gram matrix on 8 Trainium2 NeuronCores.

out[i, j] = exp(-gamma * ||x_i - y_j||^2),  x, y: [8192, 256] fp32.

Strategy (data-parallel over rows of x; y replicated):
  - Rows of x sharded across 8 cores (1024 rows each). Each core computes its
    [1024, 8192] stripe as
        out = exp(2g*(x.y) - g*||x||^2) * exp(-g*||y||^2)
    PE does the k=256 fp16 GEMM (2 k-tiles, 4 PSUM banks per 2048-col group),
    ACT applies exp with the per-partition -g*||x||^2 bias straight out of
    PSUM (fp16 out), DVE multiplies by the per-column exp(-g*||y||^2) factor
    (fp16 tensor_tensor, 2x mode), DMA streams the fp16 stripe to DRAM.
  - The per-column factor arrives pre-broadcast from the host ([128, 8192]
    fp16, 2MB); its later groups load with ~15us of slack. A handful of
    throwaway k=1 matmuls at kernel start keep the PE busy so the HAM clock
    gate reaches 2.4GHz before the real GEMM begins.
  - fp16 output halves HBM write traffic; host casts back to fp32.
    Max rel err ~1.5e-3, well inside the 2e-2 gate.
"""

import numpy as np

GAMMA = 0.005
FULL_N = 8192
D = 256
N_CORES = 8
M_SHARD = FULL_N // N_CORES  # 1024 rows of x per core
P = 128
M_TILES = M_SHARD // P  # 8
GROUP = 2048  # columns of output per PSUM tile (4 banks)
BANK = 512  # fp32 columns per PSUM bank (one matmul's max free dim)
N_GROUPS = FULL_N // GROUP  # 4

# Tiles whose exp runs as a DVE polynomial instead of on the (pacing) ACT
# engine, and the degree-4 relative-minimax fit of e^u on u in [-1.6, 0.35]
# in the nested form c4*(u^4 + b3 u^3 + b2 u^2 + b1 u) + c0 (squared later).
OFF_TILES = {(1, 6), (2, 6)}
PC4, PB3, PB2, PB1, PC0 = (
    0.021148114848613835,
    6.921948830443179,
    23.353569722985046,
    47.33351452617367,
    1.000357600651235,
)

_cache = {}


def _split_sync_waits(nc, maxw=1):
    """walrus codegen rejects instructions carrying more than ~2 sync waits
    ("Too many sync wait commands"). Tile can attach many (e.g. the tail
    drain waits on every semaphore; a matmul can wait on several DMA lanes).
    Hoist the excess onto wait-only EventSemaphore instructions inserted
    just before the offender on the same engine (engines execute their
    instructions in block order, so all waits still precede the op)."""
    import concourse.mybir as mybir

    n_new = 0
    for fn in nc.m.functions:
        for bb in fn.blocks:
            insts = bb.instructions
            if not any(
                i.sync_info is not None and len(i.sync_info.on_wait) > maxw
                for i in insts
            ):
                continue
            new = []
            for inst in insts:
                si = inst.sync_info
                if si is not None and len(si.on_wait) > maxw:
                    waits = list(si.on_wait)
                    for i in range(0, len(waits) - maxw, maxw):
                        ev = mybir.InstEventSemaphore(
                            name=f"wsplit_{n_new}", ins=[], outs=[]
                        )
                        n_new += 1
                        ev.engine = inst.engine
                        ev.sync_info = mybir.SyncInfo(
                            on_wait=waits[i : i + maxw], on_update=[]
                        )
                        new.append(ev)
                    si.on_wait = waits[len(waits) - maxw :]
                new.append(inst)
            bb.instructions = new


def _build():
    import concourse.bass as bass
    import concourse.mybir as mybir
    import concourse.tile as tile

    f32 = mybir.dt.float32
    f16 = mybir.dt.float16
    alu = mybir.AluOpType
    nc = bass.Bass("TRN2", target_bir_lowering=False, debug=False)
    xt = nc.dram_tensor("xt", [D, M_SHARD], f16, kind="ExternalInput").ap()
    yt = nc.dram_tensor("yt", [D, FULL_N], f16, kind="ExternalInput").ap()
    # cols 0..M_TILES-1: -g*||x||^2 (ACT exp bias); cols M_TILES..: half of
    # that (bias for the DVE polynomial-exp path, which works on s/2).
    x2 = nc.dram_tensor("x2", [P, 2 * M_TILES], f32, kind="ExternalInput").ap()
    eyr = nc.dram_tensor("eyr", [1, FULL_N], f16, kind="ExternalInput").ap()
    out = nc.dram_tensor("out", [M_SHARD, FULL_N], f16, kind="ExternalOutput").ap()

    with tile.TileContext(nc) as tc:
        with (
            tc.tile_pool(name="const", bufs=1) as cpool,
            tc.tile_pool(name="actp", bufs=4) as apool,
            tc.tile_pool(name="outp", bufs=6) as opool,
            tc.tile_pool(name="poly", bufs=1) as spool,
            tc.tile_pool(name="psum", bufs=2, space="PSUM") as ppool,
        ):
            ones = cpool.tile([1, P], f16, tag="ones")
            nc.any.memset(ones, 1.0)
            wrow = cpool.tile([1, BANK], f16, tag="wrow")
            nc.any.memset(wrow, 1.0)
            # Input loads, most-urgent first: the first PSUM group needs xt
            # and the first 2048 columns of both y k-tiles (as 1024-col
            # halves so the first matmuls start as soon as each lands).
            xt0 = cpool.tile([P, M_SHARD], f16, tag="xt0")
            xt1 = cpool.tile([P, M_SHARD], f16, tag="xt1")
            x2sb = cpool.tile([P, 2 * M_TILES], f32, tag="x2")
            yt0 = cpool.tile([P, FULL_N], f16, tag="yt0")
            yt1 = cpool.tile([P, FULL_N], f16, tag="yt1")
            ey = cpool.tile([P, FULL_N], f16, tag="ey")
            HALF = GROUP // 2
            # Dual-queue input issue: dma_start costs ~600ns of issue time
            # and early transfers barely overlap when serialized on one
            # queue. SP and ACT both have hardware DGE rings, and the ACT
            # engine is idle until ~13us - so the gating loads for the first
            # output tile (split into 1024-col halves) go out on both queues
            # in parallel. ey uses DMAs with a broadcast (partition-stride-0)
            # DRAM source re-reading a 16KB hot row; group g is only needed
            # by that group's DVE mults, ~15us/group into the pipeline.
            # The whole gating set for the first tiles (both xt k-tiles -
            # the PE schedule interleaves the d-phases of consecutive tiles,
            # so xt1 gates the first ACTIVATE just as much as xt0 - plus the
            # first 2048 columns of both yt k-tiles) is split into 128KB
            # chunks alternated across both issue queues: a single dma_start
            # only reaches ~60GB/s, so landing it fast needs many transfers
            # in flight.
            qs = [nc.sync, nc.scalar]
            chunks = []
            for b in range(2):
                chunks.append((xt[b * P : (b + 1) * P, :], [xt0, xt1][b]))
            gating = []
            for src, dst in chunks:
                for c in range(2):
                    csl = slice(c * (M_SHARD // 2), (c + 1) * (M_SHARD // 2))
                    gating.append((dst[:, csl], src[:, csl]))
            for b, ytd in ((0, yt0), (1, yt1)):
                for q in range(4):
                    qsl = slice(q * BANK, (q + 1) * BANK)
                    gating.append((ytd[:, qsl], yt[b * P : (b + 1) * P, qsl]))
            for i, (dst, src) in enumerate(gating):
                qs[i % 2].dma_start(out=dst, in_=src)
            nc.sync.dma_start(out=x2sb, in_=x2)
            nc.sync.dma_start(
                out=ey[:, 0:GROUP], in_=eyr[:, 0:GROUP].to_broadcast((P, GROUP))
            )
            # Preload the ACT exp table set (~1.3us) off the critical path:
            # the first real ACTIVATE would otherwise trigger it lazily.
            tldw = cpool.tile([1, 1], f16, tag="tldw")
            nc.scalar.activation(
                tldw, ones[:, 0:1], mybir.ActivationFunctionType.Exp
            )
            # Bulk loads all on the SP queue: issuing them from the ACT queue
            # risks DGE ring-credit waits that block ACTIVATEs behind them.
            for g in range(1, N_GROUPS):
                sl = slice(g * GROUP, (g + 1) * GROUP)
                nc.sync.dma_start(out=yt0[:, sl], in_=yt[0:P, sl])
                nc.sync.dma_start(out=yt1[:, sl], in_=yt[P : 2 * P, sl])
            for g in range(1, N_GROUPS):
                sl = slice(g * GROUP, (g + 1) * GROUP)
                nc.sync.dma_start(
                    out=ey[:, sl], in_=eyr[:, sl].to_broadcast((P, GROUP))
                )

            # HAM warmup: only MATMUL activity flips the PE clock gate from
            # 1.2GHz to 2.4GHz (takes a ~3.4us-busy window). These junk k=1
            # matmuls (never read; WAW into a rotating psum tile is safe)
            # depend only on the memsets, so the PE is busy from ~8us and
            # hands off to the first real matmuls as their inputs land.
            # 4 x 512-col then 8 x 128-col, so the handoff to real work is
            # fine-grained once inputs land.
            psw = ppool.tile([P, GROUP], f32, tag="ps")
            for w in range(4):
                nc.tensor.matmul(
                    psw[:, 0:BANK], ones, wrow, start=True, stop=True
                )
            for w in range(6):
                nc.tensor.matmul(
                    psw[:, 0:P], ones, wrow[:, 0:P], start=True, stop=True
                )

            # Main loop, g-major. The last tile's mult+store run in 1024-col
            # halves to shorten the drain tail.
            for g in range(N_GROUPS):
                for t in range(M_TILES):
                    last = g == N_GROUPS - 1 and t == M_TILES - 1
                    msl = slice(t * P, (t + 1) * P)
                    ps = ppool.tile([P, GROUP], f32, tag="ps")
                    for d, (xtd, ytd) in enumerate(((xt0, yt0), (xt1, yt1))):
                        for b in range(GROUP // BANK):
                            nsl = slice(
                                g * GROUP + b * BANK,
                                g * GROUP + (b + 1) * BANK,
                            )
                            bsl = slice(b * BANK, (b + 1) * BANK)
                            nc.tensor.matmul(
                                ps[:, bsl], xtd[:, msl], ytd[:, nsl],
                                start=(d == 0), stop=(d == 1),
                            )
                    if (g, t) in OFF_TILES:
                        # Polynomial-exp path on the DVE (which has ~20us of
                        # slack while ACT paces the kernel): u = s/2, then
                        # e^s ~= (c4*(u^4+b3 u^3+b2 u^2+b1 u)+c0)^2, nested
                        # as three (acc+b)*u steps. Adds ~6e-3 rel err on
                        # these tiles, well inside the 2e-2 gate.
                        gsl = slice(g * GROUP, (g + 1) * GROUP)
                        x2h = x2sb[:, M_TILES + t : M_TILES + t + 1]
                        u = spool.tile([P, GROUP], f16, tag="pu")
                        nc.vector.tensor_scalar(
                            u, ps, GAMMA, x2h, op0=alu.mult, op1=alu.add
                        )
                        h1 = spool.tile([P, GROUP], f16, tag="ph1")
                        nc.vector.scalar_tensor_tensor(
                            h1, u, PB3, u, op0=alu.add, op1=alu.mult
                        )
                        h2 = spool.tile([P, GROUP], f16, tag="ph2")
                        nc.vector.scalar_tensor_tensor(
                            h2, h1, PB2, u, op0=alu.add, op1=alu.mult
                        )
                        h3 = spool.tile([P, GROUP], f16, tag="ph1")
                        nc.vector.scalar_tensor_tensor(
                            h3, h2, PB1, u, op0=alu.add, op1=alu.mult
                        )
                        pp = spool.tile([P, GROUP], f16, tag="ph2")
                        nc.vector.tensor_scalar(
                            pp, h3, PC4, PC0, op0=alu.mult, op1=alu.add
                        )
                        sq = spool.tile([P, GROUP], f16, tag="pu")
                        nc.vector.tensor_mul(sq, pp, pp)
                        ot = opool.tile([P, GROUP], f16, tag="ot")
                        nc.vector.tensor_mul(ot, sq, ey[:, gsl])
                        nc.sync.dma_start(out=out[msl, gsl], in_=ot)
                        continue
                    at = apool.tile([P, GROUP], f16, tag="at")
                    ot = opool.tile([P, GROUP], f16, tag="ot")
                    if last:
                        pieces = [
                            (b * BANK, (b + 1) * BANK)
                            for b in range(GROUP // BANK)
                        ]
                    else:
                        pieces = [(0, GROUP)]
                    for pi, (h0, h1) in enumerate(pieces):
                        hsl = slice(h0, h1)
                        ngsl = slice(g * GROUP + h0, g * GROUP + h1)
                        # exp(2g*(x.y) - g*||x||^2): bias is per-partition,
                        # free on the ACT datapath.
                        nc.scalar.activation(
                            at[:, hsl], ps[:, hsl],
                            mybir.ActivationFunctionType.Exp,
                            bias=x2sb[:, t : t + 1], scale=2.0 * GAMMA,
                        )
                        nc.vector.tensor_mul(
                            ot[:, hsl], at[:, hsl], ey[:, ngsl]
                        )
                        # The final stores alternate queues so the two last
                        # transfers run in parallel.
                        q = qs[pi % 2] if last else nc.sync
                        q.dma_start(out=out[msl, ngsl], in_=ot[:, hsl])

    _split_sync_waits(nc, maxw=1)
    return nc


def kernel(x: np.ndarray, y: np.ndarray) -> np.ndarray:
    from concourse import bass_utils

    x = np.asarray(x, dtype=np.float32)
    y = np.asarray(y, dtype=np.float32)

    if "nc" not in _cache:
        _cache["nc"] = _build()
    nc = _cache["nc"]

    yt = np.ascontiguousarray(y.T.astype(np.float16))  # [256, 8192]
    xt_full = x.T.astype(np.float16)  # [256, 8192]
    x2 = np.sum(x.astype(np.float64) * x.astype(np.float64), axis=1)  # [8192]
    y2 = np.sum(y.astype(np.float64) * y.astype(np.float64), axis=1)
    eyr = np.exp(-GAMMA * y2).astype(np.float16).reshape(1, FULL_N)

    in_maps = []
    for c in range(N_CORES):
        cols = slice(c * M_SHARD, (c + 1) * M_SHARD)
        x2c = (-GAMMA * x2[cols]).astype(np.float32)
        x2t = x2c.reshape(M_TILES, P).T  # [P, M_TILES]
        in_maps.append(
            {
                "xt": np.ascontiguousarray(xt_full[:, cols]),
                "yt": yt,
                "x2": np.ascontiguousarray(
                    np.concatenate([x2t, 0.5 * x2t], axis=1)
                ),
                "eyr": eyr,
            }
        )

    res = bass_utils.run_bass_kernel_spmd(
        nc, in_maps, core_ids=list(range(N_CORES))
    )
    _cache["last_result"] = res
    return np.concatenate(
        [res.results[c]["out"] for c in range(N_CORES)], axis=0
    ).astype(np.float32)


# revision 44
# speedup vs baseline: 1.2381x; 1.2381x over previous
"""RBF (Gaussian) kernel Gram matrix on 8 Trainium2 NeuronCores.

out[i, j] = exp(-gamma * ||x_i - y_j||^2),  x, y: [8192, 256] fp32.

Strategy (data-parallel over rows of x; y replicated):
  - Rows of x sharded across 8 cores (1024 rows each). Each core computes its
    [1024, 8192] stripe as
        out = exp(2g*(x.y) - g*||x||^2) * exp(-g*||y||^2)
    PE does the k=256 fp16 GEMM (2 k-tiles, 4 PSUM banks per 2048-col group),
    ACT applies exp with the per-partition -g*||x||^2 bias straight out of
    PSUM (fp16 out), DVE multiplies by the per-column exp(-g*||y||^2) factor
    (fp16 tensor_tensor, 2x mode), DMA streams the fp16 stripe to DRAM.
  - The per-column factor arrives pre-broadcast from the host ([128, 8192]
    fp16, 2MB); its later groups load with ~15us of slack. A handful of
    throwaway k=1 matmuls at kernel start keep the PE busy so the HAM clock
    gate reaches 2.4GHz before the real GEMM begins.
  - fp16 output halves HBM write traffic; host casts back to fp32.
    Max rel err ~1.5e-3, well inside the 2e-2 gate.
"""

import numpy as np

GAMMA = 0.005
FULL_N = 8192
D = 256
N_CORES = 8
M_SHARD = FULL_N // N_CORES  # 1024 rows of x per core
P = 128
M_TILES = M_SHARD // P  # 8
GROUP = 2048  # columns of output per PSUM tile (4 banks)
BANK = 512  # fp32 columns per PSUM bank (one matmul's max free dim)
N_GROUPS = FULL_N // GROUP  # 4

# Tiles whose exp runs as a DVE polynomial instead of on the (pacing) ACT
# engine, and the degree-4 relative-minimax fit of e^u on u in [-1.6, 0.35]
# in the nested form c4*(u^4 + b3 u^3 + b2 u^2 + b1 u) + c0 (squared later).
OFF_TILES = set()  # {(1, 6), (2, 6)}: the DVE has the capacity but its
# strict-FIFO queue makes the poly burst block later tiles' mults,
# stalling the ACT stream - net loss. Kept for reference.
PC4, PB3, PB2, PB1, PC0 = (
    0.021148114848613835,
    6.921948830443179,
    23.353569722985046,
    47.33351452617367,
    1.000357600651235,
)

_cache = {}


def _split_sync_waits(nc, maxw=1):
    """walrus codegen rejects instructions carrying more than ~2 sync waits
    ("Too many sync wait commands"). Tile can attach many (e.g. the tail
    drain waits on every semaphore; a matmul can wait on several DMA lanes).
    Hoist the excess onto wait-only EventSemaphore instructions inserted
    just before the offender on the same engine (engines execute their
    instructions in block order, so all waits still precede the op)."""
    import concourse.mybir as mybir

    n_new = 0
    for fn in nc.m.functions:
        for bb in fn.blocks:
            insts = bb.instructions
            if not any(
                i.sync_info is not None and len(i.sync_info.on_wait) > maxw
                for i in insts
            ):
                continue
            new = []
            for inst in insts:
                si = inst.sync_info
                if si is not None and len(si.on_wait) > maxw:
                    waits = list(si.on_wait)
                    for i in range(0, len(waits) - maxw, maxw):
                        ev = mybir.InstEventSemaphore(
                            name=f"wsplit_{n_new}", ins=[], outs=[]
                        )
                        n_new += 1
                        ev.engine = inst.engine
                        ev.sync_info = mybir.SyncInfo(
                            on_wait=waits[i : i + maxw], on_update=[]
                        )
                        new.append(ev)
                    si.on_wait = waits[len(waits) - maxw :]
                new.append(inst)
            bb.instructions = new


def _build():
    import concourse.bass as bass
    import concourse.mybir as mybir
    import concourse.tile as tile

    f32 = mybir.dt.float32
    f16 = mybir.dt.float16
    alu = mybir.AluOpType
    nc = bass.Bass("TRN2", target_bir_lowering=False, debug=False)
    xt = nc.dram_tensor("xt", [D, M_SHARD], f16, kind="ExternalInput").ap()
    yt = nc.dram_tensor("yt", [D, FULL_N], f16, kind="ExternalInput").ap()
    # cols 0..M_TILES-1: -g*||x||^2 (ACT exp bias); cols M_TILES..: half of
    # that (bias for the DVE polynomial-exp path, which works on s/2).
    x2 = nc.dram_tensor("x2", [P, 2 * M_TILES], f32, kind="ExternalInput").ap()
    eyr = nc.dram_tensor("eyr", [1, FULL_N], f16, kind="ExternalInput").ap()
    out = nc.dram_tensor("out", [M_SHARD, FULL_N], f16, kind="ExternalOutput").ap()

    with tile.TileContext(nc) as tc:
        with (
            tc.tile_pool(name="const", bufs=1) as cpool,
            tc.tile_pool(name="actp", bufs=4) as apool,
            tc.tile_pool(name="outp", bufs=6) as opool,
            tc.tile_pool(name="poly", bufs=1) as spool,
            tc.tile_pool(name="psum", bufs=2, space="PSUM") as ppool,
        ):
            ones = cpool.tile([1, P], f16, tag="ones")
            nc.any.memset(ones, 1.0)
            wrow = cpool.tile([1, BANK], f16, tag="wrow")
            nc.any.memset(wrow, 1.0)
            # Input loads, most-urgent first: the first PSUM group needs xt
            # and the first 2048 columns of both y k-tiles (as 1024-col
            # halves so the first matmuls start as soon as each lands).
            xt0 = cpool.tile([P, M_SHARD], f16, tag="xt0")
            xt1 = cpool.tile([P, M_SHARD], f16, tag="xt1")
            x2sb = cpool.tile([P, 2 * M_TILES], f32, tag="x2")
            yt0 = cpool.tile([P, FULL_N], f16, tag="yt0")
            yt1 = cpool.tile([P, FULL_N], f16, tag="yt1")
            ey = cpool.tile([P, FULL_N], f16, tag="ey")
            HALF = GROUP // 2
            # Dual-queue input issue: dma_start costs ~600ns of issue time
            # and early transfers barely overlap when serialized on one
            # queue. SP and ACT both have hardware DGE rings, and the ACT
            # engine is idle until ~13us - so the gating loads for the first
            # output tile (split into 1024-col halves) go out on both queues
            # in parallel. ey uses DMAs with a broadcast (partition-stride-0)
            # DRAM source re-reading a 16KB hot row; group g is only needed
            # by that group's DVE mults, ~15us/group into the pipeline.
            # The whole gating set for the first tiles (both xt k-tiles -
            # the PE schedule interleaves the d-phases of consecutive tiles,
            # so xt1 gates the first ACTIVATE just as much as xt0 - plus the
            # first 2048 columns of both yt k-tiles) is split into 128KB
            # chunks alternated across both issue queues: a single dma_start
            # only reaches ~60GB/s, so landing it fast needs many transfers
            # in flight.
            qs = [nc.sync, nc.scalar]
            chunks = []
            for b in range(2):
                chunks.append((xt[b * P : (b + 1) * P, :], [xt0, xt1][b]))
            gating = []
            for src, dst in chunks:
                for c in range(2):
                    csl = slice(c * (M_SHARD // 2), (c + 1) * (M_SHARD // 2))
                    gating.append((dst[:, csl], src[:, csl]))
            for b, ytd in ((0, yt0), (1, yt1)):
                for q in range(4):
                    qsl = slice(q * BANK, (q + 1) * BANK)
                    gating.append((ytd[:, qsl], yt[b * P : (b + 1) * P, qsl]))
            for i, (dst, src) in enumerate(gating):
                qs[i % 2].dma_start(out=dst, in_=src)
            nc.sync.dma_start(out=x2sb, in_=x2)
            nc.sync.dma_start(
                out=ey[:, 0:GROUP], in_=eyr[:, 0:GROUP].to_broadcast((P, GROUP))
            )
            # Preload the ACT exp table set (~1.3us) off the critical path:
            # the first real ACTIVATE would otherwise trigger it lazily.
            tldw = cpool.tile([1, 1], f16, tag="tldw")
            nc.scalar.activation(
                tldw, ones[:, 0:1], mybir.ActivationFunctionType.Exp
            )
            # Bulk loads all on the SP queue: issuing them from the ACT queue
            # risks DGE ring-credit waits that block ACTIVATEs behind them.
            for g in range(1, N_GROUPS):
                sl = slice(g * GROUP, (g + 1) * GROUP)
                nc.sync.dma_start(out=yt0[:, sl], in_=yt[0:P, sl])
                nc.sync.dma_start(out=yt1[:, sl], in_=yt[P : 2 * P, sl])
            for g in range(1, N_GROUPS):
                sl = slice(g * GROUP, (g + 1) * GROUP)
                nc.sync.dma_start(
                    out=ey[:, sl], in_=eyr[:, sl].to_broadcast((P, GROUP))
                )

            # HAM warmup: only MATMUL activity flips the PE clock gate from
            # 1.2GHz to 2.4GHz (takes a ~3.4us-busy window). These junk k=1
            # matmuls (never read; WAW into a rotating psum tile is safe)
            # depend only on the memsets, so the PE is busy from ~8us and
            # hands off to the first real matmuls as their inputs land.
            # 4 x 512-col then 8 x 128-col, so the handoff to real work is
            # fine-grained once inputs land.
            psw = ppool.tile([P, GROUP], f32, tag="ps")
            for w in range(4):
                nc.tensor.matmul(
                    psw[:, 0:BANK], ones, wrow, start=True, stop=True
                )
            for w in range(6):
                nc.tensor.matmul(
                    psw[:, 0:P], ones, wrow[:, 0:P], start=True, stop=True
                )

            # Main loop, g-major. The last tile's mult+store run in 1024-col
            # halves to shorten the drain tail.
            for g in range(N_GROUPS):
                for t in range(M_TILES):
                    last = g == N_GROUPS - 1 and t == M_TILES - 1
                    msl = slice(t * P, (t + 1) * P)
                    ps = ppool.tile([P, GROUP], f32, tag="ps")
                    for d, (xtd, ytd) in enumerate(((xt0, yt0), (xt1, yt1))):
                        for b in range(GROUP // BANK):
                            nsl = slice(
                                g * GROUP + b * BANK,
                                g * GROUP + (b + 1) * BANK,
                            )
                            bsl = slice(b * BANK, (b + 1) * BANK)
                            nc.tensor.matmul(
                                ps[:, bsl], xtd[:, msl], ytd[:, nsl],
                                start=(d == 0), stop=(d == 1),
                            )
                    if (g, t) in OFF_TILES:
                        # Polynomial-exp path on the DVE (which has ~20us of
                        # slack while ACT paces the kernel): u = s/2, then
                        # e^s ~= (c4*(u^4+b3 u^3+b2 u^2+b1 u)+c0)^2, nested
                        # as three (acc+b)*u steps. Adds ~6e-3 rel err on
                        # these tiles, well inside the 2e-2 gate.
                        gsl = slice(g * GROUP, (g + 1) * GROUP)
                        x2h = x2sb[:, M_TILES + t : M_TILES + t + 1]
                        u = spool.tile([P, GROUP], f16, tag="pu")
                        nc.vector.tensor_scalar(
                            u, ps, GAMMA, x2h, op0=alu.mult, op1=alu.add
                        )
                        h1 = spool.tile([P, GROUP], f16, tag="ph1")
                        nc.vector.scalar_tensor_tensor(
                            h1, u, PB3, u, op0=alu.add, op1=alu.mult
                        )
                        h2 = spool.tile([P, GROUP], f16, tag="ph2")
                        nc.vector.scalar_tensor_tensor(
                            h2, h1, PB2, u, op0=alu.add, op1=alu.mult
                        )
                        h3 = spool.tile([P, GROUP], f16, tag="ph1")
                        nc.vector.scalar_tensor_tensor(
                            h3, h2, PB1, u, op0=alu.add, op1=alu.mult
                        )
                        pp = spool.tile([P, GROUP], f16, tag="ph2")
                        nc.vector.tensor_scalar(
                            pp, h3, PC4, PC0, op0=alu.mult, op1=alu.add
                        )
                        sq = spool.tile([P, GROUP], f16, tag="pu")
                        nc.vector.tensor_mul(sq, pp, pp)
                        ot = opool.tile([P, GROUP], f16, tag="ot")
                        nc.vector.tensor_mul(ot, sq, ey[:, gsl])
                        nc.sync.dma_start(out=out[msl, gsl], in_=ot)
                        continue
                    at = apool.tile([P, GROUP], f16, tag="at")
                    ot = opool.tile([P, GROUP], f16, tag="ot")
                    if last:
                        pieces = [
                            (b * BANK, (b + 1) * BANK)
                            for b in range(GROUP // BANK)
                        ]
                    else:
                        pieces = [(0, GROUP)]
                    for pi, (h0, h1) in enumerate(pieces):
                        hsl = slice(h0, h1)
                        ngsl = slice(g * GROUP + h0, g * GROUP + h1)
                        # exp(2g*(x.y) - g*||x||^2): bias is per-partition,
                        # free on the ACT datapath.
                        nc.scalar.activation(
                            at[:, hsl], ps[:, hsl],
                            mybir.ActivationFunctionType.Exp,
                            bias=x2sb[:, t : t + 1], scale=2.0 * GAMMA,
                        )
                        nc.vector.tensor_mul(
                            ot[:, hsl], at[:, hsl], ey[:, ngsl]
                        )
                        # The final stores alternate queues so the two last
                        # transfers run in parallel.
                        q = qs[pi % 2] if last else nc.sync
                        q.dma_start(out=out[msl, ngsl], in_=ot[:, hsl])

    _split_sync_waits(nc, maxw=1)
    return nc


def kernel(x: np.ndarray, y: np.ndarray) -> np.ndarray:
    from concourse import bass_utils

    x = np.asarray(x, dtype=np.float32)
    y = np.asarray(y, dtype=np.float32)

    if "nc" not in _cache:
        _cache["nc"] = _build()
    nc = _cache["nc"]

    yt = np.ascontiguousarray(y.T.astype(np.float16))  # [256, 8192]
    xt_full = x.T.astype(np.float16)  # [256, 8192]
    x2 = np.sum(x.astype(np.float64) * x.astype(np.float64), axis=1)  # [8192]
    y2 = np.sum(y.astype(np.float64) * y.astype(np.float64), axis=1)
    eyr = np.exp(-GAMMA * y2).astype(np.float16).reshape(1, FULL_N)

    in_maps = []
    for c in range(N_CORES):
        cols = slice(c * M_SHARD, (c + 1) * M_SHARD)
        x2c = (-GAMMA * x2[cols]).astype(np.float32)
        x2t = x2c.reshape(M_TILES, P).T  # [P, M_TILES]
        in_maps.append(
            {
                "xt": np.ascontiguousarray(xt_full[:, cols]),
                "yt": yt,
                "x2": np.ascontiguousarray(
                    np.concatenate([x2t, 0.5 * x2t], axis=1)
                ),
                "eyr": eyr,
            }
        )

    res = bass_utils.run_bass_kernel_spmd(
        nc, in_maps, core_ids=list(range(N_CORES))
    )
    _cache["last_result"] = res
    return np.concatenate(
        [res.results[c]["out"] for c in range(N_CORES)], axis=0
    ).astype(np.float32)
